# revision 20
# baseline (speedup 1.0000x reference)
"""Trainium2 Bass kernel for nn_AddSLoss (retrieval_knn).

Per batch b:
  tf = model_points @ base + t  (rigid transform, base = H[:3,:3]^T, t = H[:3,3])
  d2[n,m] = |tf[n] - target[m]|^2 ; inds = argmin_m d2
  dis[b] = mean_n |tf[n] - target[inds[n]]| if idx[b] symmetric else
           mean_n |tf[n] - target[n]|

Key identity: |tf[n]-target[argmin]| == sqrt(min_m d2[n,m]) — no argmin
index or gather needed, only a min-reduction over the d2 matrix.

Sharding: pure data parallel, 4 batches per core on 8 cores.

The d2 matrix is produced by TensorE as one K=13 bf16 matmul per tile with
split-precision (hi/lo) rows so the result carries ~fp32 accuracy:
  d2[n,m] = q2[n] + (r2[m] - 2 t.target[m]) - 2 tf[n].target[m]
  lhsT rows: hi(tf)x3, lo(tf)x3, hi(tf)x3 again, hi(q2), lo(q2), 1, 1
  rhs rows:  hi(-2tg)x3, hi(-2tg)x3, lo(-2tg)x3, 1, 1, hi(r3), lo(r3)
VectorE does the min-reduction over [128,1024] PSUM tiles (the bottleneck),
plus the natural-layout prep (tf, q2, r2, t.tgt, direct distances).
Row-layout operands are assembled via PE transpose + ACT copy + SBUF DMA.
"""

import functools
import numpy as np

BS, NP, NCORES, BPC, P = 32, 2048, 8, 4, 128
KR = 13  # contraction rows of the score matmul
DVE_QTS = frozenset({0})  # qt min-reduced directly from PSUM on DVE


@functools.lru_cache(maxsize=1)
def _build():
    import concourse.bacc as bacc
    import concourse.mybir as mybir
    import concourse.tile as tile

    f32 = mybir.dt.float32
    bf16 = mybir.dt.bfloat16
    AX = mybir.AxisListType.X
    OP = mybir.AluOpType
    AF = mybir.ActivationFunctionType

    nc = bacc.Bacc(
        "TRN2",
        target_bir_lowering=False,
        debug=False,
        enable_asserts=False,
        num_devices=NCORES,
    )

    # rhsA[b]: bf16 rows 0-2 hi(-2*tgt_e), 3-5 hi(-2*tgt_e), 6-8 lo(-2*tgt_e),
    #          9-10 ones. (rows 11-12 = hi/lo of r2-2*t.tgt are device-built)
    rhsA = nc.dram_tensor("rhsA", [BPC, 11, NP], bf16, kind="ExternalInput")
    # pk2[b] f32, planar natural q-major layout (col 16*c + q, point 128q+p):
    #   cols 0:48 mp_d ; 48:96 tgt_e ; 96:144 t_e bcast ; 144:153 base[d,e]
    pk2 = nc.dram_tensor("pk2", [BPC, P, 160], f32, kind="ExternalInput")
    idn = nc.dram_tensor("idn", [P, P], bf16, kind="ExternalInput")
    onesr = nc.dram_tensor("onesr", [2, NP], bf16, kind="ExternalInput")
    wv = nc.dram_tensor("wv", [P, 1], f32, kind="ExternalInput")
    out = nc.dram_tensor("out", [1, 8], f32, kind="ExternalOutput")

    with tile.TileContext(nc) as tc:
        with (
            tc.tile_pool(name="cst", bufs=1) as cst,
            tc.tile_pool(name="io", bufs=2) as io,
            tc.tile_pool(name="wk", bufs=2) as wk,
            tc.tile_pool(name="acc", bufs=1) as acc,
            tc.tile_pool(name="ps", bufs=2, space="PSUM") as ps,
        ):
            ident = cst.tile([P, P], bf16, tag="ident")
            nc.sync.dma_start(ident[:], idn.ap())
            wvec = cst.tile([P, 1], f32, tag="wvec")
            nc.sync.dma_start(wvec[:], wv.ap())
            ones2 = cst.tile([2, NP], bf16, tag="ones2")
            nc.sync.dma_start(ones2[:], onesr.ap())
            dall = acc.tile([P, P], f32, tag="dall")  # d^2 collection pre-sqrt

            for b in range(BPC):
                rhs = wk.tile([KR, NP], bf16, tag="rhs")
                nc.sync.dma_start(rhs[0:11, :], rhsA.ap()[b])
                t2 = io.tile([P, 160], f32, tag="t2")
                nc.sync.dma_start(t2[:], pk2.ap()[b])

                mpN = t2[:, 0:48]
                tgN = t2[:, 48:96]
                tbc = t2[:, 96:144]
                bbc = t2[:, 144:153]

                def byq(ap):  # [p, 48] -> [p, q(16), c(3)] (c strided by 16)
                    return ap.rearrange("p (c q) -> p q c", c=3)

                # ---- natural-layout prep (planar blocks of 16 cols per coord)
                sc1 = wk.tile([P, 48], f32, tag="sc1")
                nc.vector.tensor_tensor(sc1[:], tgN, tgN, OP.mult)
                r2n = wk.tile([P, 16], f32, tag="r2n")
                nc.vector.tensor_reduce(r2n[:], byq(sc1[:]), AX, OP.add)
                # tf_e = mp_x*B[0,e] + mp_y*B[1,e] + mp_z*B[2,e] + t_e (planar)
                tfn = wk.tile([P, 48], f32, tag="tfn")
                s1 = wk.tile([P, 16], f32, tag="s1")
                s2 = wk.tile([P, 16], f32, tag="s2")
                for e in range(3):
                    nc.vector.scalar_tensor_tensor(
                        s1[:], mpN[:, 0:16], bbc[:, e : e + 1],
                        tbc[:, 16 * e : 16 * e + 16], OP.mult, OP.add,
                    )
                    nc.vector.scalar_tensor_tensor(
                        s2[:], mpN[:, 16:32], bbc[:, 3 + e : 4 + e],
                        s1[:], OP.mult, OP.add,
                    )
                    nc.vector.scalar_tensor_tensor(
                        tfn[:, 16 * e : 16 * e + 16], mpN[:, 32:48],
                        bbc[:, 6 + e : 7 + e], s2[:], OP.mult, OP.add,
                    )
                sc3 = wk.tile([P, 48], f32, tag="sc3")
                nc.vector.tensor_tensor(sc3[:], tfn[:], tfn[:], OP.mult)
                q2n = wk.tile([P, 16], f32, tag="q2n")
                nc.vector.tensor_reduce(q2n[:], byq(sc3[:]), AX, OP.add)

                # direct |tf - tgt|^2 -> dall cols 64+
                df = wk.tile([P, 48], f32, tag="df")
                nc.vector.tensor_tensor(df[:], tfn[:], tgN, OP.subtract)
                dfs = wk.tile([P, 48], f32, tag="dfs")
                nc.vector.tensor_tensor(dfs[:], df[:], df[:], OP.mult)
                dd2 = wk.tile([P, 16], f32, tag="dd2")
                nc.vector.tensor_reduce(dd2[:], byq(dfs[:]), AX, OP.add)
                nc.vector.tensor_scalar_max(
                    dall[:, 64 + b * 16 : 64 + (b + 1) * 16], dd2[:], 0.0
                )

                # ---- hi/lo splits (bf16) for the matmul operand rows
                tf6 = wk.tile([P, 96], bf16, tag="tf6")
                nc.vector.tensor_copy(tf6[:, 0:48], tfn[:])
                nc.vector.tensor_tensor(tf6[:, 48:96], tfn[:], tf6[:, 0:48],
                                        OP.subtract)
                q4 = wk.tile([P, 64], bf16, tag="q4")
                nc.vector.tensor_copy(q4[:, 0:16], q2n[:])
                nc.vector.tensor_tensor(q4[:, 16:32], q2n[:], q4[:, 0:16],
                                        OP.subtract)
                nc.vector.tensor_copy(q4[:, 32:48], r2n[:])
                nc.vector.tensor_tensor(q4[:, 48:64], r2n[:], q4[:, 32:48],
                                        OP.subtract)

                # ---- transpose to row layout; assemble lhsT
                psA = ps.tile([P, 128], bf16, tag="ps")
                nc.tensor.transpose(psA[0:96, :], tf6[:], ident[:])
                tA = wk.tile([96, P], bf16, tag="tA")
                nc.scalar.copy(tA[:], psA[0:96, :])
                psB = ps.tile([P, 128], bf16, tag="ps")
                nc.tensor.transpose(psB[0:64, :], q4[:], ident[:])
                tB = wk.tile([64, P], bf16, tag="tB")
                nc.scalar.copy(tB[:], psB[0:64, :])

                lhsT = wk.tile([KR, NP], bf16, tag="lhsT")

                def rowify(ap, nr):  # [nr,NP] dest as [nr,16,128]
                    return ap.rearrange("p (q c) -> p q c", q=16)

                nc.sync.dma_start(rowify(lhsT[0:3, :], 3), tA[0:48, :])
                nc.sync.dma_start(rowify(lhsT[3:6, :], 3), tA[48:96, :])
                nc.sync.dma_start(rowify(lhsT[6:9, :], 3), tA[0:48, :])
                nc.sync.dma_start(rowify(lhsT[9:11, :], 2), tB[0:32, :])
                nc.sync.dma_start(lhsT[11:13, :], ones2[:])
                nc.sync.dma_start(rowify(rhs[11:13, :], 2), tB[32:64, :])

                # ---- score matmuls (PSUM = d2 directly) + min reduction.
                # qt in DVE_QTS: direct f32 reduce_min from PSUM on VectorE.
                # other qt: ACT casts PSUM->SBUF bf16, VectorE runs a 2x-rate
                # bf16 tensor_tensor min tree (halves DVE cost per tile).
                minh = wk.tile([P, 16], f32, tag="minh")
                for qt in range(16):
                    lhs = lhsT[:, qt * P : (qt + 1) * P]
                    direct = qt in DVE_QTS
                    s_ps = ps.tile([P, 2048], f32, tag="ps")
                    for j in range(4):
                        nc.tensor.matmul(
                            s_ps[:, j * 512 : (j + 1) * 512],
                            lhs,
                            rhs[:, j * 512 : (j + 1) * 512],
                            start=True,
                            stop=True,
                        )
                    if direct:
                        nc.vector.tensor_reduce(
                            minh[:, qt : qt + 1], s_ps[:], AX, OP.min
                        )
                    else:
                        sb = wk.tile([P, 2048], bf16, tag="sbq")
                        nc.scalar.copy(sb[:], s_ps[:])
                        l1 = wk.tile([P, 1024], bf16, tag="lv1")
                        nc.vector.tensor_tensor(
                            l1[:], sb[:, 0:1024], sb[:, 1024:2048], OP.min
                        )
                        l2 = wk.tile([P, 512], bf16, tag="lv2")
                        nc.vector.tensor_tensor(
                            l2[:], l1[:, 0:512], l1[:, 512:1024], OP.min
                        )
                        l3 = wk.tile([P, 256], bf16, tag="lv3")
                        nc.vector.tensor_tensor(
                            l3[:], l2[:, 0:256], l2[:, 256:512], OP.min
                        )
                        l4 = wk.tile([P, 128], bf16, tag="lv4")
                        nc.vector.tensor_tensor(
                            l4[:], l3[:, 0:128], l3[:, 128:256], OP.min
                        )
                        nc.vector.tensor_reduce(
                            minh[:, qt : qt + 1], l4[:], AX, OP.min
                        )
                nc.vector.tensor_scalar_max(
                    dall[:, b * 16 : (b + 1) * 16], minh[:], 0.0
                )

            # ---- sqrt then mean over the 2048 points of each batch
            sq = acc.tile([P, P], f32, tag="sq")
            nc.scalar.activation(sq[:], dall[:], AF.Sqrt)
            o_ps = ps.tile([P, 128], f32, tag="ps")
            nc.tensor.matmul(o_ps[0:1, 0:P], wvec[:], sq[:], start=True, stop=True)
            osb = acc.tile([1, 8], f32, tag="osb")
            nc.vector.tensor_reduce(
                osb[:], o_ps[0:1, 0:P].rearrange("p (a c) -> p a c", c=16), AX, OP.add
            )
            nc.sync.dma_start(out.ap(), osb[:])

    nc.compile()
    return nc


def _host_prep(target, model_points, idx, H):
    """Per-core input maps. Layout/dtype transforms + O(B) H/idx math only."""
    import ml_dtypes

    bf = ml_dtypes.bfloat16
    tgt = np.ascontiguousarray(target, dtype=np.float32)
    mp = np.ascontiguousarray(model_points, dtype=np.float32)
    Hf = np.ascontiguousarray(H, dtype=np.float32)

    # rhs host rows: hi/lo split of -2*target coords (planar rows)
    t2 = (-2.0 * tgt).transpose(0, 2, 1)  # [BS, 3, NP]
    t2hi = t2.astype(bf)
    t2lo = (t2 - t2hi.astype(np.float32)).astype(bf)
    rhsA = np.zeros((BS, 11, NP), dtype=bf)
    rhsA[:, 0:3] = t2hi
    rhsA[:, 3:6] = t2hi
    rhsA[:, 6:9] = t2lo
    rhsA[:, 9:11] = np.ones((1, 2, NP), dtype=bf)

    # planar natural q-major layout: col 16c+q holds point 128q+p, channel c
    def nat(x):  # [BS, NP, 3] -> [BS, P, 48] planar
        return (
            x.reshape(BS, 16, P, 3).transpose(0, 2, 3, 1).reshape(BS, P, 48)
        )

    pk2 = np.zeros((BS, P, 160), dtype=np.float32)
    pk2[:, :, 0:48] = nat(mp)
    pk2[:, :, 48:96] = nat(tgt)
    t_vec = Hf[:, :3, 3]
    pk2[:, :, 96:144] = np.repeat(t_vec, 16, axis=1)[:, None, :]
    # bbc col 3d+e = base[d,e] = H[e,d]
    bb = Hf[:, :3, :3].transpose(0, 2, 1).reshape(BS, 9)
    pk2[:, :, 144:153] = bb[:, None, :]

    idn = np.eye(P, dtype=np.float32).astype(bf)
    onesr = np.ones((2, NP), dtype=bf)
    wvv = np.full((P, 1), 1.0 / NP, dtype=np.float32)

    in_maps = []
    for c in range(NCORES):
        sl = slice(c * BPC, (c + 1) * BPC)
        in_maps.append(
            {
                "rhsA": np.ascontiguousarray(rhsA[sl]),
                "pk2": np.ascontiguousarray(pk2[sl]),
                "idn": idn,
                "onesr": onesr,
                "wv": wvv,
            }
        )
    is_sym = (np.asarray(idx)[:, 0] % 2 == 0) & (np.asarray(idx)[:, 0] < 16)
    return in_maps, is_sym


def run_on_device(target, model_points, idx, H, trace=False, **kw):
    from concourse import bass_utils

    nc = _build()
    in_maps, is_sym = _host_prep(target, model_points, idx, H)
    res = bass_utils.run_bass_kernel_spmd(
        nc, in_maps, core_ids=list(range(NCORES)), trace=trace, **kw
    )
    dis = np.zeros((BS,), dtype=np.float32)
    for c in range(NCORES):
        o = res.results[c]["out"].reshape(8)
        for bb_ in range(BPC):
            g = c * BPC + bb_
            dis[g] = o[bb_] if is_sym[g] else o[4 + bb_]
    return dis, res


def kernel(target, model_points, idx, H):
    dis, _ = run_on_device(target, model_points, idx, H)
    return dis


# revision 21
# speedup vs baseline: 1.0095x; 1.0095x over previous
"""Trainium2 Bass kernel for nn_AddSLoss (retrieval_knn).

Per batch b:
  tf = model_points @ base + t  (rigid transform, base = H[:3,:3]^T, t = H[:3,3])
  d2[n,m] = |tf[n] - target[m]|^2 ; inds = argmin_m d2
  dis[b] = mean_n |tf[n] - target[inds[n]]| if idx[b] symmetric else
           mean_n |tf[n] - target[n]|

Key identity: |tf[n]-target[argmin]| == sqrt(min_m d2[n,m]) — no argmin
index or gather needed, only a min-reduction over the d2 matrix.

Sharding: pure data parallel, 4 batches per core on 8 cores.

The d2 matrix is produced by TensorE as one K=13 bf16 matmul per tile with
split-precision (hi/lo) rows so the result carries ~fp32 accuracy:
  d2[n,m] = q2[n] + (r2[m] - 2 t.target[m]) - 2 tf[n].target[m]
  lhsT rows: hi(tf)x3, lo(tf)x3, hi(tf)x3 again, hi(q2), lo(q2), 1, 1
  rhs rows:  hi(-2tg)x3, hi(-2tg)x3, lo(-2tg)x3, 1, 1, hi(r3), lo(r3)
VectorE does the min-reduction over [128,1024] PSUM tiles (the bottleneck),
plus the natural-layout prep (tf, q2, r2, t.tgt, direct distances).
Row-layout operands are assembled via PE transpose + ACT copy + SBUF DMA.
"""

import functools
import numpy as np

BS, NP, NCORES, BPC, P = 32, 2048, 8, 4, 128
KR = 13  # contraction rows of the score matmul
DVE_QTS = frozenset()  # qt min-reduced directly from PSUM on DVE


@functools.lru_cache(maxsize=1)
def _build():
    import concourse.bacc as bacc
    import concourse.mybir as mybir
    import concourse.tile as tile

    f32 = mybir.dt.float32
    bf16 = mybir.dt.bfloat16
    AX = mybir.AxisListType.X
    OP = mybir.AluOpType
    AF = mybir.ActivationFunctionType

    nc = bacc.Bacc(
        "TRN2",
        target_bir_lowering=False,
        debug=False,
        enable_asserts=False,
        num_devices=NCORES,
    )

    # rhsA[b]: bf16 rows 0-2 hi(-2*tgt_e), 3-5 hi(-2*tgt_e), 6-8 lo(-2*tgt_e),
    #          9-10 ones. (rows 11-12 = hi/lo of r2-2*t.tgt are device-built)
    rhsA = nc.dram_tensor("rhsA", [BPC, 11, NP], bf16, kind="ExternalInput")
    # pk2[b] f32, planar natural q-major layout (col 16*c + q, point 128q+p):
    #   cols 0:48 mp_d ; 48:96 tgt_e ; 96:144 t_e bcast ; 144:153 base[d,e]
    pk2 = nc.dram_tensor("pk2", [BPC, P, 160], f32, kind="ExternalInput")
    idn = nc.dram_tensor("idn", [P, P], bf16, kind="ExternalInput")
    onesr = nc.dram_tensor("onesr", [2, NP], bf16, kind="ExternalInput")
    wv = nc.dram_tensor("wv", [P, 1], f32, kind="ExternalInput")
    out = nc.dram_tensor("out", [1, 8], f32, kind="ExternalOutput")

    with tile.TileContext(nc) as tc:
        with (
            tc.tile_pool(name="cst", bufs=1) as cst,
            tc.tile_pool(name="io", bufs=2) as io,
            tc.tile_pool(name="wk", bufs=2) as wk,
            tc.tile_pool(name="acc", bufs=1) as acc,
            tc.tile_pool(name="ps", bufs=2, space="PSUM") as ps,
        ):
            ident = cst.tile([P, P], bf16, tag="ident")
            nc.sync.dma_start(ident[:], idn.ap())
            wvec = cst.tile([P, 1], f32, tag="wvec")
            nc.sync.dma_start(wvec[:], wv.ap())
            ones2 = cst.tile([2, NP], bf16, tag="ones2")
            nc.sync.dma_start(ones2[:], onesr.ap())
            dall = acc.tile([P, P], f32, tag="dall")  # d^2 collection pre-sqrt

            for b in range(BPC):
                rhs = wk.tile([KR, NP], bf16, tag="rhs")
                nc.sync.dma_start(rhs[0:11, :], rhsA.ap()[b])
                t2 = io.tile([P, 160], f32, tag="t2")
                nc.sync.dma_start(t2[:], pk2.ap()[b])

                mpN = t2[:, 0:48]
                tgN = t2[:, 48:96]
                tbc = t2[:, 96:144]
                bbc = t2[:, 144:153]

                def byq(ap):  # [p, 48] -> [p, q(16), c(3)] (c strided by 16)
                    return ap.rearrange("p (c q) -> p q c", c=3)

                # ---- natural-layout prep (planar blocks of 16 cols per coord)
                sc1 = wk.tile([P, 48], f32, tag="sc1")
                nc.vector.tensor_tensor(sc1[:], tgN, tgN, OP.mult)
                r2n = wk.tile([P, 16], f32, tag="r2n")
                nc.vector.tensor_reduce(r2n[:], byq(sc1[:]), AX, OP.add)
                # tf_e = mp_x*B[0,e] + mp_y*B[1,e] + mp_z*B[2,e] + t_e (planar)
                tfn = wk.tile([P, 48], f32, tag="tfn")
                s1 = wk.tile([P, 16], f32, tag="s1")
                s2 = wk.tile([P, 16], f32, tag="s2")
                for e in range(3):
                    nc.vector.scalar_tensor_tensor(
                        s1[:], mpN[:, 0:16], bbc[:, e : e + 1],
                        tbc[:, 16 * e : 16 * e + 16], OP.mult, OP.add,
                    )
                    nc.vector.scalar_tensor_tensor(
                        s2[:], mpN[:, 16:32], bbc[:, 3 + e : 4 + e],
                        s1[:], OP.mult, OP.add,
                    )
                    nc.vector.scalar_tensor_tensor(
                        tfn[:, 16 * e : 16 * e + 16], mpN[:, 32:48],
                        bbc[:, 6 + e : 7 + e], s2[:], OP.mult, OP.add,
                    )
                sc3 = wk.tile([P, 48], f32, tag="sc3")
                nc.vector.tensor_tensor(sc3[:], tfn[:], tfn[:], OP.mult)
                q2n = wk.tile([P, 16], f32, tag="q2n")
                nc.vector.tensor_reduce(q2n[:], byq(sc3[:]), AX, OP.add)

                # direct |tf - tgt|^2 -> dall cols 64+
                df = wk.tile([P, 48], f32, tag="df")
                nc.vector.tensor_tensor(df[:], tfn[:], tgN, OP.subtract)
                dfs = wk.tile([P, 48], f32, tag="dfs")
                nc.vector.tensor_tensor(dfs[:], df[:], df[:], OP.mult)
                dd2 = wk.tile([P, 16], f32, tag="dd2")
                nc.vector.tensor_reduce(dd2[:], byq(dfs[:]), AX, OP.add)
                nc.vector.tensor_scalar_max(
                    dall[:, 64 + b * 16 : 64 + (b + 1) * 16], dd2[:], 0.0
                )

                # ---- hi/lo splits (bf16) for the matmul operand rows
                tf6 = wk.tile([P, 96], bf16, tag="tf6")
                nc.vector.tensor_copy(tf6[:, 0:48], tfn[:])
                nc.vector.tensor_tensor(tf6[:, 48:96], tfn[:], tf6[:, 0:48],
                                        OP.subtract)
                q4 = wk.tile([P, 64], bf16, tag="q4")
                nc.vector.tensor_copy(q4[:, 0:16], q2n[:])
                nc.vector.tensor_tensor(q4[:, 16:32], q2n[:], q4[:, 0:16],
                                        OP.subtract)
                nc.vector.tensor_copy(q4[:, 32:48], r2n[:])
                nc.vector.tensor_tensor(q4[:, 48:64], r2n[:], q4[:, 32:48],
                                        OP.subtract)

                # ---- transpose to row layout; assemble lhsT
                psA = ps.tile([P, 128], bf16, tag="ps")
                nc.tensor.transpose(psA[0:96, :], tf6[:], ident[:])
                tA = wk.tile([96, P], bf16, tag="tA")
                nc.scalar.copy(tA[:], psA[0:96, :])
                psB = ps.tile([P, 128], bf16, tag="ps")
                nc.tensor.transpose(psB[0:64, :], q4[:], ident[:])
                tB = wk.tile([64, P], bf16, tag="tB")
                nc.scalar.copy(tB[:], psB[0:64, :])

                lhsT = wk.tile([KR, NP], bf16, tag="lhsT")

                def rowify(ap, nr):  # [nr,NP] dest as [nr,16,128]
                    return ap.rearrange("p (q c) -> p q c", q=16)

                nc.sync.dma_start(rowify(lhsT[0:3, :], 3), tA[0:48, :])
                nc.sync.dma_start(rowify(lhsT[3:6, :], 3), tA[48:96, :])
                nc.sync.dma_start(rowify(lhsT[6:9, :], 3), tA[0:48, :])
                nc.sync.dma_start(rowify(lhsT[9:11, :], 2), tB[0:32, :])
                nc.sync.dma_start(lhsT[11:13, :], ones2[:])
                nc.sync.dma_start(rowify(rhs[11:13, :], 2), tB[32:64, :])

                # ---- score matmuls (PSUM = d2 directly) + min reduction.
                # qt in DVE_QTS: direct f32 reduce_min from PSUM on VectorE.
                # other qt: ACT casts PSUM->SBUF bf16, VectorE runs a 2x-rate
                # bf16 tensor_tensor min tree (halves DVE cost per tile).
                minh = wk.tile([P, 16], f32, tag="minh")
                l1p = None
                l2q = None
                for qt in range(16):
                    lhs = lhsT[:, qt * P : (qt + 1) * P]
                    s_ps = ps.tile([P, 2048], f32, tag="ps")
                    for j in range(4):
                        nc.tensor.matmul(
                            s_ps[:, j * 512 : (j + 1) * 512],
                            lhs,
                            rhs[:, j * 512 : (j + 1) * 512],
                            start=True,
                            stop=True,
                        )
                    if qt in DVE_QTS:
                        nc.vector.tensor_reduce(
                            minh[:, qt : qt + 1], s_ps[:], AX, OP.min
                        )
                        continue
                    sb = wk.tile([P, 2048], bf16, tag="sbq")
                    nc.scalar.copy(sb[:], s_ps[:])
                    # lvl1 per qt: 2048 -> 1024, pair-packed
                    if qt % 2 == 0:
                        l1p = wk.tile([P, 2048], bf16, tag="l1p")
                    nc.vector.tensor_tensor(
                        l1p[:, (qt % 2) * 1024 : (qt % 2 + 1) * 1024],
                        sb[:, 0:1024], sb[:, 1024:2048], OP.min,
                    )
                    if qt % 2 == 1:
                        # lvl2 batched over the qt pair: -> [128, 2, 512]
                        pi = (qt // 2) % 2
                        if pi == 0:
                            l2q = wk.tile([P, 2048], bf16, tag="l2q")
                        v1 = l1p[:].rearrange("p (a c) -> p a c", a=2)
                        nc.vector.tensor_tensor(
                            l2q[:, pi * 1024 : (pi + 1) * 1024].rearrange(
                                "p (a c) -> p a c", a=2
                            ),
                            v1[:, :, 0:512], v1[:, :, 512:1024], OP.min,
                        )
                    if qt % 4 == 3:
                        # lvl3/lvl4/final reduce batched over the qt quad
                        v2 = l2q[:].rearrange("p (a c) -> p a c", a=4)
                        l3 = wk.tile([P, 1024], bf16, tag="l3")
                        nc.vector.tensor_tensor(
                            l3[:].rearrange("p (a c) -> p a c", a=4),
                            v2[:, :, 0:256], v2[:, :, 256:512], OP.min,
                        )
                        v3 = l3[:].rearrange("p (a c) -> p a c", a=4)
                        l4 = wk.tile([P, 512], bf16, tag="l4")
                        nc.vector.tensor_tensor(
                            l4[:].rearrange("p (a c) -> p a c", a=4),
                            v3[:, :, 0:128], v3[:, :, 128:256], OP.min,
                        )
                        nc.vector.tensor_reduce(
                            minh[:, qt - 3 : qt + 1],
                            l4[:].rearrange("p (a c) -> p a c", a=4),
                            AX, OP.min,
                        )
                nc.vector.tensor_scalar_max(
                    dall[:, b * 16 : (b + 1) * 16], minh[:], 0.0
                )

            # ---- sqrt then mean over the 2048 points of each batch
            sq = acc.tile([P, P], f32, tag="sq")
            nc.scalar.activation(sq[:], dall[:], AF.Sqrt)
            o_ps = ps.tile([P, 128], f32, tag="ps")
            nc.tensor.matmul(o_ps[0:1, 0:P], wvec[:], sq[:], start=True, stop=True)
            osb = acc.tile([1, 8], f32, tag="osb")
            nc.vector.tensor_reduce(
                osb[:], o_ps[0:1, 0:P].rearrange("p (a c) -> p a c", c=16), AX, OP.add
            )
            nc.sync.dma_start(out.ap(), osb[:])

    nc.compile()
    return nc


def _host_prep(target, model_points, idx, H):
    """Per-core input maps. Layout/dtype transforms + O(B) H/idx math only."""
    import ml_dtypes

    bf = ml_dtypes.bfloat16
    tgt = np.ascontiguousarray(target, dtype=np.float32)
    mp = np.ascontiguousarray(model_points, dtype=np.float32)
    Hf = np.ascontiguousarray(H, dtype=np.float32)

    # rhs host rows: hi/lo split of -2*target coords (planar rows)
    t2 = (-2.0 * tgt).transpose(0, 2, 1)  # [BS, 3, NP]
    t2hi = t2.astype(bf)
    t2lo = (t2 - t2hi.astype(np.float32)).astype(bf)
    rhsA = np.zeros((BS, 11, NP), dtype=bf)
    rhsA[:, 0:3] = t2hi
    rhsA[:, 3:6] = t2hi
    rhsA[:, 6:9] = t2lo
    rhsA[:, 9:11] = np.ones((1, 2, NP), dtype=bf)

    # planar natural q-major layout: col 16c+q holds point 128q+p, channel c
    def nat(x):  # [BS, NP, 3] -> [BS, P, 48] planar
        return (
            x.reshape(BS, 16, P, 3).transpose(0, 2, 3, 1).reshape(BS, P, 48)
        )

    pk2 = np.zeros((BS, P, 160), dtype=np.float32)
    pk2[:, :, 0:48] = nat(mp)
    pk2[:, :, 48:96] = nat(tgt)
    t_vec = Hf[:, :3, 3]
    pk2[:, :, 96:144] = np.repeat(t_vec, 16, axis=1)[:, None, :]
    # bbc col 3d+e = base[d,e] = H[e,d]
    bb = Hf[:, :3, :3].transpose(0, 2, 1).reshape(BS, 9)
    pk2[:, :, 144:153] = bb[:, None, :]

    idn = np.eye(P, dtype=np.float32).astype(bf)
    onesr = np.ones((2, NP), dtype=bf)
    wvv = np.full((P, 1), 1.0 / NP, dtype=np.float32)

    in_maps = []
    for c in range(NCORES):
        sl = slice(c * BPC, (c + 1) * BPC)
        in_maps.append(
            {
                "rhsA": np.ascontiguousarray(rhsA[sl]),
                "pk2": np.ascontiguousarray(pk2[sl]),
                "idn": idn,
                "onesr": onesr,
                "wv": wvv,
            }
        )
    is_sym = (np.asarray(idx)[:, 0] % 2 == 0) & (np.asarray(idx)[:, 0] < 16)
    return in_maps, is_sym


def run_on_device(target, model_points, idx, H, trace=False, **kw):
    from concourse import bass_utils

    nc = _build()
    in_maps, is_sym = _host_prep(target, model_points, idx, H)
    res = bass_utils.run_bass_kernel_spmd(
        nc, in_maps, core_ids=list(range(NCORES)), trace=trace, **kw
    )
    dis = np.zeros((BS,), dtype=np.float32)
    for c in range(NCORES):
        o = res.results[c]["out"].reshape(8)
        for bb_ in range(BPC):
            g = c * BPC + bb_
            dis[g] = o[bb_] if is_sym[g] else o[4 + bb_]
    return dis, res


def kernel(target, model_points, idx, H):
    dis, _ = run_on_device(target, model_points, idx, H)
    return dis


# revision 24
# speedup vs baseline: 1.0845x; 1.0742x over previous
"""Trainium2 Bass kernel for nn_AddSLoss (retrieval_knn).

Per batch b:
  tf = model_points @ base + t  (rigid transform, base = H[:3,:3]^T, t = H[:3,3])
  d2[n,m] = |tf[n] - target[m]|^2 ; inds = argmin_m d2
  dis[b] = mean_n |tf[n] - target[inds[n]]| if idx[b] symmetric else
           mean_n |tf[n] - target[n]|

Key identity: |tf[n]-target[argmin]| == sqrt(min_m d2[n,m]) — no argmin
index or gather needed, only a min-reduction over the d2 matrix.

Sharding: pure data parallel, 4 batches per core on 8 cores.

The d2 matrix is produced by TensorE as one K=13 bf16 matmul per tile with
split-precision (hi/lo) rows so the result carries ~fp32 accuracy:
  d2[n,m] = q2[n] + (r2[m] - 2 t.target[m]) - 2 tf[n].target[m]
  lhsT rows: hi(tf)x3, lo(tf)x3, hi(tf)x3 again, hi(q2), lo(q2), 1, 1
  rhs rows:  hi(-2tg)x3, hi(-2tg)x3, lo(-2tg)x3, 1, 1, hi(r3), lo(r3)
VectorE does the min-reduction over [128,1024] PSUM tiles (the bottleneck),
plus the natural-layout prep (tf, q2, r2, t.tgt, direct distances).
Row-layout operands are assembled via PE transpose + ACT copy + SBUF DMA.
"""

import functools
import numpy as np

BS, NP, NCORES, BPC, P = 32, 2048, 8, 4, 128
KR = 13  # contraction rows of the score matmul
DVE_QTS = frozenset({12, 13})  # qt min-reduced directly from PSUM on DVE


@functools.lru_cache(maxsize=1)
def _build():
    import concourse.bacc as bacc
    import concourse.mybir as mybir
    import concourse.tile as tile

    f32 = mybir.dt.float32
    bf16 = mybir.dt.bfloat16
    AX = mybir.AxisListType.X
    OP = mybir.AluOpType
    AF = mybir.ActivationFunctionType

    nc = bacc.Bacc(
        "TRN2",
        target_bir_lowering=False,
        debug=False,
        enable_asserts=False,
        num_devices=NCORES,
    )

    # rhsA[b]: bf16 rows 0-2 hi(-2*tgt_e), 3-5 hi(-2*tgt_e), 6-8 lo(-2*tgt_e),
    #          9-10 ones. (rows 11-12 = hi/lo of r2-2*t.tgt are device-built)
    rhsA = nc.dram_tensor("rhsA", [BPC, 11, NP], bf16, kind="ExternalInput")
    # pk2[b] f32, planar natural q-major layout (col 16*c + q, point 128q+p):
    #   cols 0:48 mp_d ; 48:96 tgt_e ; 96:144 t_e bcast ; 144:153 base[d,e]
    pk2 = nc.dram_tensor("pk2", [BPC, P, 160], f32, kind="ExternalInput")
    onesr = nc.dram_tensor("onesr", [2, NP], bf16, kind="ExternalInput")
    wv = nc.dram_tensor("wv", [P, 1], f32, kind="ExternalInput")
    out = nc.dram_tensor("out", [1, 8], f32, kind="ExternalOutput")

    with tile.TileContext(nc) as tc:
        with (
            tc.tile_pool(name="cst", bufs=1) as cst,
            tc.tile_pool(name="io", bufs=2) as io,
            tc.tile_pool(name="wk", bufs=2) as wk,
            tc.tile_pool(name="acc", bufs=1) as acc,
            tc.tile_pool(name="ps", bufs=2, space="PSUM") as ps,
        ):
            wvec = cst.tile([P, 1], f32, tag="wvec")
            nc.sync.dma_start(wvec[:], wv.ap())
            ones2 = cst.tile([2, NP], bf16, tag="ones2")
            nc.sync.dma_start(ones2[:], onesr.ap())
            dall = acc.tile([P, P], f32, tag="dall")  # d^2 collection pre-sqrt

            def byq(ap):  # [p, 48] -> [p, q(16), c(3)] (c strided by 16)
                return ap.rearrange("p (c q) -> p q c", c=3)

            def rowify(ap):  # [nr, NP] dest viewed as [nr, 16, 128]
                return ap.rearrange("p (q c) -> p q c", q=16)

            def prep(b):
                """Everything for batch b that does not depend on the score
                matmuls: loads, natural-layout math, hi/lo splits, operand
                assembly via DMA xbar transpose, and the direct-distance path.
                """
                rhs = wk.tile([KR, NP], bf16, tag="rhs")
                nc.sync.dma_start(rhs[0:11, :], rhsA.ap()[b])
                t2 = io.tile([P, 160], f32, tag="t2")
                nc.sync.dma_start(t2[:], pk2.ap()[b])

                mpN = t2[:, 0:48]
                tgN = t2[:, 48:96]
                tbc = t2[:, 96:144]
                bbc = t2[:, 144:153]

                # natural-layout prep (planar blocks of 16 cols per coord)
                sc1 = wk.tile([P, 48], f32, tag="sc1")
                nc.vector.tensor_tensor(sc1[:], tgN, tgN, OP.mult)
                r2n = wk.tile([P, 16], f32, tag="r2n")
                nc.vector.tensor_reduce(r2n[:], byq(sc1[:]), AX, OP.add)
                # tf_e = mp_x*B[0,e] + mp_y*B[1,e] + mp_z*B[2,e] + t_e (planar)
                tfn = wk.tile([P, 48], f32, tag="tfn")
                s1 = wk.tile([P, 16], f32, tag="s1")
                s2 = wk.tile([P, 16], f32, tag="s2")
                for e in range(3):
                    nc.vector.scalar_tensor_tensor(
                        s1[:], mpN[:, 0:16], bbc[:, e : e + 1],
                        tbc[:, 16 * e : 16 * e + 16], OP.mult, OP.add,
                    )
                    nc.vector.scalar_tensor_tensor(
                        s2[:], mpN[:, 16:32], bbc[:, 3 + e : 4 + e],
                        s1[:], OP.mult, OP.add,
                    )
                    nc.vector.scalar_tensor_tensor(
                        tfn[:, 16 * e : 16 * e + 16], mpN[:, 32:48],
                        bbc[:, 6 + e : 7 + e], s2[:], OP.mult, OP.add,
                    )
                sc3 = wk.tile([P, 48], f32, tag="sc3")
                nc.vector.tensor_tensor(sc3[:], tfn[:], tfn[:], OP.mult)
                q2n = wk.tile([P, 16], f32, tag="q2n")
                nc.vector.tensor_reduce(q2n[:], byq(sc3[:]), AX, OP.add)

                # direct |tf - tgt|^2 -> dall cols 64+
                df = wk.tile([P, 48], f32, tag="df")
                nc.vector.tensor_tensor(df[:], tfn[:], tgN, OP.subtract)
                dfs = wk.tile([P, 48], f32, tag="dfs")
                nc.vector.tensor_tensor(dfs[:], df[:], df[:], OP.mult)
                dd2 = wk.tile([P, 16], f32, tag="dd2")
                nc.vector.tensor_reduce(dd2[:], byq(dfs[:]), AX, OP.add)
                nc.vector.tensor_scalar_max(
                    dall[:, 64 + b * 16 : 64 + (b + 1) * 16], dd2[:], 0.0
                )

                # hi/lo splits, packed for the DMA xbar transpose
                x1 = wk.tile([P, 128], bf16, tag="x1")
                nc.vector.tensor_copy(x1[:, 0:48], tfn[:])
                nc.vector.tensor_tensor(x1[:, 48:96], tfn[:], x1[:, 0:48],
                                        OP.subtract)
                nc.vector.tensor_copy(x1[:, 96:112], q2n[:])
                nc.vector.tensor_tensor(x1[:, 112:128], q2n[:], x1[:, 96:112],
                                        OP.subtract)
                x2 = wk.tile([P, 128], bf16, tag="x2")
                nc.vector.tensor_copy(x2[:, 0:16], r2n[:])
                nc.vector.tensor_tensor(x2[:, 16:32], r2n[:], x2[:, 0:16],
                                        OP.subtract)

                x1t = wk.tile([P, 128], bf16, tag="x1t")
                nc.sync.dma_start(x1t[:], x1[:], transpose=True)
                x2t = wk.tile([P, 128], bf16, tag="x2t")
                nc.sync.dma_start(x2t[:], x2[:], transpose=True)

                lhsT = wk.tile([KR, NP], bf16, tag="lhsT")
                nc.sync.dma_start(rowify(lhsT[0:3, :]), x1t[0:48, :])
                nc.sync.dma_start(rowify(lhsT[3:6, :]), x1t[48:96, :])
                nc.sync.dma_start(rowify(lhsT[6:9, :]), x1t[0:48, :])
                nc.sync.dma_start(rowify(lhsT[9:11, :]), x1t[96:128, :])
                nc.sync.dma_start(lhsT[11:13, :], ones2[:])
                nc.sync.dma_start(rowify(rhs[11:13, :]), x2t[0:32, :])
                return lhsT, rhs

            def scores(b, lhsT, rhs):
                minh = wk.tile([P, 16], f32, tag="minh")
                l1p = None
                l2q = None
                ti = 0  # index among tree-path qts (pair/quad bookkeeping)
                pend = []  # first qt of the pending pair/quad
                for qt in range(16):
                    lhs = lhsT[:, qt * P : (qt + 1) * P]
                    s_ps = ps.tile([P, 2048], f32, tag="ps")
                    for j in range(4):
                        nc.tensor.matmul(
                            s_ps[:, j * 512 : (j + 1) * 512],
                            lhs,
                            rhs[:, j * 512 : (j + 1) * 512],
                            start=True,
                            stop=True,
                        )
                    if qt in DVE_QTS:
                        nc.vector.tensor_reduce(
                            minh[:, qt : qt + 1], s_ps[:], AX, OP.min
                        )
                        continue
                    sb = wk.tile([P, 2048], bf16, tag="sbq")
                    nc.scalar.copy(sb[:], s_ps[:])
                    # lvl1 per qt: 2048 -> 1024, pair-packed
                    slot = ti % 2
                    if slot == 0:
                        l1p = wk.tile([P, 2048], bf16, tag="l1p")
                        pend.append(qt)
                    nc.vector.tensor_tensor(
                        l1p[:, slot * 1024 : (slot + 1) * 1024],
                        sb[:, 0:1024], sb[:, 1024:2048], OP.min,
                    )
                    if slot == 1:
                        # lvl2 batched over the qt pair -> two 512 blocks
                        blk = (ti // 2) % 2
                        if blk == 0:
                            l2q = wk.tile([P, 2048], bf16, tag="l2q")
                        v1 = l1p[:].rearrange("p (a c) -> p a c", a=2)
                        nc.vector.tensor_tensor(
                            l2q[:, blk * 1024 : (blk + 1) * 1024].rearrange(
                                "p (a c) -> p a c", a=2
                            ),
                            v1[:, :, 0:512], v1[:, :, 512:1024], OP.min,
                        )
                        if blk == 1:
                            # lvl3/lvl4/final reduce batched over the quad;
                            # tree qts of this quad are contiguous (q0..q0+3)
                            q0 = pend[0]
                            v2 = l2q[:].rearrange("p (a c) -> p a c", a=4)
                            l3 = wk.tile([P, 1024], bf16, tag="l3")
                            nc.vector.tensor_tensor(
                                l3[:].rearrange("p (a c) -> p a c", a=4),
                                v2[:, :, 0:256], v2[:, :, 256:512], OP.min,
                            )
                            v3 = l3[:].rearrange("p (a c) -> p a c", a=4)
                            l4 = wk.tile([P, 512], bf16, tag="l4")
                            nc.vector.tensor_tensor(
                                l4[:].rearrange("p (a c) -> p a c", a=4),
                                v3[:, :, 0:128], v3[:, :, 128:256], OP.min,
                            )
                            nc.vector.tensor_reduce(
                                minh[:, q0 : q0 + 4],
                                l4[:].rearrange("p (a c) -> p a c", a=4),
                                AX, OP.min,
                            )
                            pend = []
                    ti += 1
                if pend:
                    # leftover pair: lvl3/lvl4/reduce at pair granularity
                    q0 = pend[0]
                    v2 = l2q[:, 0:1024].rearrange("p (a c) -> p a c", a=2)
                    l3 = wk.tile([P, 1024], bf16, tag="l3")
                    nc.vector.tensor_tensor(
                        l3[:, 0:512].rearrange("p (a c) -> p a c", a=2),
                        v2[:, :, 0:256], v2[:, :, 256:512], OP.min,
                    )
                    v3 = l3[:, 0:512].rearrange("p (a c) -> p a c", a=2)
                    l4 = wk.tile([P, 512], bf16, tag="l4")
                    nc.vector.tensor_tensor(
                        l4[:, 0:256].rearrange("p (a c) -> p a c", a=2),
                        v3[:, :, 0:128], v3[:, :, 128:256], OP.min,
                    )
                    nc.vector.tensor_reduce(
                        minh[:, q0 : q0 + 2],
                        l4[:, 0:256].rearrange("p (a c) -> p a c", a=2),
                        AX, OP.min,
                    )
                nc.vector.tensor_scalar_max(
                    dall[:, b * 16 : (b + 1) * 16], minh[:], 0.0
                )

            handles = prep(0)
            for b in range(BPC):
                nxt = prep(b + 1) if b + 1 < BPC else None
                scores(b, *handles)
                handles = nxt

            # ---- sqrt then mean over the 2048 points of each batch
            sq = acc.tile([P, P], f32, tag="sq")
            nc.scalar.activation(sq[:], dall[:], AF.Sqrt)
            o_ps = ps.tile([P, 128], f32, tag="ps")
            nc.tensor.matmul(o_ps[0:1, 0:P], wvec[:], sq[:], start=True, stop=True)
            osb = acc.tile([1, 8], f32, tag="osb")
            nc.vector.tensor_reduce(
                osb[:], o_ps[0:1, 0:P].rearrange("p (a c) -> p a c", c=16), AX, OP.add
            )
            nc.sync.dma_start(out.ap(), osb[:])

    nc.compile()
    return nc


def _host_prep(target, model_points, idx, H):
    """Per-core input maps. Layout/dtype transforms + O(B) H/idx math only."""
    import ml_dtypes

    bf = ml_dtypes.bfloat16
    tgt = np.ascontiguousarray(target, dtype=np.float32)
    mp = np.ascontiguousarray(model_points, dtype=np.float32)
    Hf = np.ascontiguousarray(H, dtype=np.float32)

    # rhs host rows: hi/lo split of -2*target coords (planar rows)
    t2 = (-2.0 * tgt).transpose(0, 2, 1)  # [BS, 3, NP]
    t2hi = t2.astype(bf)
    t2lo = (t2 - t2hi.astype(np.float32)).astype(bf)
    rhsA = np.zeros((BS, 11, NP), dtype=bf)
    rhsA[:, 0:3] = t2hi
    rhsA[:, 3:6] = t2hi
    rhsA[:, 6:9] = t2lo
    rhsA[:, 9:11] = np.ones((1, 2, NP), dtype=bf)

    # planar natural q-major layout: col 16c+q holds point 128q+p, channel c
    def nat(x):  # [BS, NP, 3] -> [BS, P, 48] planar
        return (
            x.reshape(BS, 16, P, 3).transpose(0, 2, 3, 1).reshape(BS, P, 48)
        )

    pk2 = np.zeros((BS, P, 160), dtype=np.float32)
    pk2[:, :, 0:48] = nat(mp)
    pk2[:, :, 48:96] = nat(tgt)
    t_vec = Hf[:, :3, 3]
    pk2[:, :, 96:144] = np.repeat(t_vec, 16, axis=1)[:, None, :]
    # bbc col 3d+e = base[d,e] = H[e,d]
    bb = Hf[:, :3, :3].transpose(0, 2, 1).reshape(BS, 9)
    pk2[:, :, 144:153] = bb[:, None, :]

    onesr = np.ones((2, NP), dtype=bf)
    wvv = np.full((P, 1), 1.0 / NP, dtype=np.float32)

    in_maps = []
    for c in range(NCORES):
        sl = slice(c * BPC, (c + 1) * BPC)
        in_maps.append(
            {
                "rhsA": np.ascontiguousarray(rhsA[sl]),
                "pk2": np.ascontiguousarray(pk2[sl]),
                "onesr": onesr,
                "wv": wvv,
            }
        )
    is_sym = (np.asarray(idx)[:, 0] % 2 == 0) & (np.asarray(idx)[:, 0] < 16)
    return in_maps, is_sym


def run_on_device(target, model_points, idx, H, trace=False, **kw):
    from concourse import bass_utils

    nc = _build()
    in_maps, is_sym = _host_prep(target, model_points, idx, H)
    res = bass_utils.run_bass_kernel_spmd(
        nc, in_maps, core_ids=list(range(NCORES)), trace=trace, **kw
    )
    dis = np.zeros((BS,), dtype=np.float32)
    for c in range(NCORES):
        o = res.results[c]["out"].reshape(8)
        for bb_ in range(BPC):
            g = c * BPC + bb_
            dis[g] = o[bb_] if is_sym[g] else o[4 + bb_]
    return dis, res


def kernel(target, model_points, idx, H):
    dis, _ = run_on_device(target, model_points, idx, H)
    return dis


# revision 26
# speedup vs baseline: 1.1320x; 1.0438x over previous
"""Trainium2 Bass kernel for nn_AddSLoss (retrieval_knn).

Per batch b:
  tf = model_points @ base + t  (rigid transform, base = H[:3,:3]^T, t = H[:3,3])
  d2[n,m] = |tf[n] - target[m]|^2 ; inds = argmin_m d2
  dis[b] = mean_n |tf[n] - target[inds[n]]| if idx[b] symmetric else
           mean_n |tf[n] - target[n]|

Key identity: |tf[n]-target[argmin]| == sqrt(min_m d2[n,m]) — no argmin
index or gather needed, only a min-reduction over the d2 matrix.

Sharding: pure data parallel, 4 batches per core on 8 cores.

The d2 matrix is produced by TensorE as one K=13 bf16 matmul per tile with
split-precision (hi/lo) rows so the result carries ~fp32 accuracy:
  d2[n,m] = q2[n] + (r2[m] - 2 t.target[m]) - 2 tf[n].target[m]
  lhsT rows: hi(tf)x3, lo(tf)x3, hi(tf)x3 again, hi(q2), lo(q2), 1, 1
  rhs rows:  hi(-2tg)x3, hi(-2tg)x3, lo(-2tg)x3, 1, 1, hi(r3), lo(r3)
VectorE does the min-reduction over [128,1024] PSUM tiles (the bottleneck),
plus the natural-layout prep (tf, q2, r2, t.tgt, direct distances).
Row-layout operands are assembled via PE transpose + ACT copy + SBUF DMA.
"""

import functools
import numpy as np

BS, NP, NCORES, BPC, P = 32, 2048, 8, 4, 128
KR = 13  # contraction rows of the score matmul
DVE_QTS = frozenset({12, 13})  # qt min-reduced directly from PSUM on DVE


@functools.lru_cache(maxsize=1)
def _build():
    import concourse.bacc as bacc
    import concourse.mybir as mybir
    import concourse.tile as tile

    f32 = mybir.dt.float32
    bf16 = mybir.dt.bfloat16
    AX = mybir.AxisListType.X
    OP = mybir.AluOpType
    AF = mybir.ActivationFunctionType

    nc = bacc.Bacc(
        "TRN2",
        target_bir_lowering=False,
        debug=False,
        enable_asserts=False,
        num_devices=NCORES,
    )

    # rhsA[b]: bf16 rows 0-2 hi(-2*tgt_e), 3-5 hi(-2*tgt_e), 6-8 lo(-2*tgt_e),
    #          9-10 ones. (rows 11-12 = hi/lo of r2-2*t.tgt are device-built)
    rhsA = nc.dram_tensor("rhsA", [BPC, 11, NP], bf16, kind="ExternalInput")
    # pk2[b] f32, planar natural q-major layout (col 16*c + q, point 128q+p):
    #   cols 0:48 mp_d ; 48:96 tgt_e ; 96:144 t_e bcast ; 144:153 base[d,e]
    pk2 = nc.dram_tensor("pk2", [BPC, P, 160], f32, kind="ExternalInput")
    onesr = nc.dram_tensor("onesr", [2, NP], bf16, kind="ExternalInput")
    wv = nc.dram_tensor("wv", [P, 1], f32, kind="ExternalInput")
    out = nc.dram_tensor("out", [1, 8], f32, kind="ExternalOutput")

    with tile.TileContext(nc) as tc:
        with (
            tc.tile_pool(name="cst", bufs=1) as cst,
            tc.tile_pool(name="io", bufs=2) as io,
            tc.tile_pool(name="wk", bufs=2) as wk,
            tc.tile_pool(name="acc", bufs=1) as acc,
            tc.tile_pool(name="ps", bufs=4, space="PSUM") as ps,
        ):
            wvec = cst.tile([P, 1], f32, tag="wvec")
            nc.sync.dma_start(wvec[:], wv.ap())
            ones2 = cst.tile([2, NP], bf16, tag="ones2")
            nc.sync.dma_start(ones2[:], onesr.ap())
            dall = acc.tile([P, P], f32, tag="dall")  # d^2 collection pre-sqrt

            def byq(ap):  # [p, 48] -> [p, q(16), c(3)] (c strided by 16)
                return ap.rearrange("p (c q) -> p q c", c=3)

            def rowify(ap):  # [nr, NP] dest viewed as [nr, 16, 128]
                return ap.rearrange("p (q c) -> p q c", q=16)

            def prep(b):
                """Everything for batch b that does not depend on the score
                matmuls: loads, natural-layout math, hi/lo splits, operand
                assembly via DMA xbar transpose, and the direct-distance path.
                """
                rhs = wk.tile([KR, NP], bf16, tag="rhs")
                nc.sync.dma_start(rhs[0:11, :], rhsA.ap()[b])
                t2 = io.tile([P, 160], f32, tag="t2")
                nc.sync.dma_start(t2[:], pk2.ap()[b])

                mpN = t2[:, 0:48]
                tgN = t2[:, 48:96]
                tbc = t2[:, 96:144]
                bbc = t2[:, 144:153]

                # natural-layout prep (planar blocks of 16 cols per coord)
                sc1 = wk.tile([P, 48], f32, tag="sc1")
                nc.vector.tensor_tensor(sc1[:], tgN, tgN, OP.mult)
                r2n = wk.tile([P, 16], f32, tag="r2n")
                nc.vector.tensor_reduce(r2n[:], byq(sc1[:]), AX, OP.add)
                # tf_e = mp_x*B[0,e] + mp_y*B[1,e] + mp_z*B[2,e] + t_e (planar)
                tfn = wk.tile([P, 48], f32, tag="tfn")
                s1 = wk.tile([P, 16], f32, tag="s1")
                s2 = wk.tile([P, 16], f32, tag="s2")
                for e in range(3):
                    nc.vector.scalar_tensor_tensor(
                        s1[:], mpN[:, 0:16], bbc[:, e : e + 1],
                        tbc[:, 16 * e : 16 * e + 16], OP.mult, OP.add,
                    )
                    nc.vector.scalar_tensor_tensor(
                        s2[:], mpN[:, 16:32], bbc[:, 3 + e : 4 + e],
                        s1[:], OP.mult, OP.add,
                    )
                    nc.vector.scalar_tensor_tensor(
                        tfn[:, 16 * e : 16 * e + 16], mpN[:, 32:48],
                        bbc[:, 6 + e : 7 + e], s2[:], OP.mult, OP.add,
                    )
                sc3 = wk.tile([P, 48], f32, tag="sc3")
                nc.vector.tensor_tensor(sc3[:], tfn[:], tfn[:], OP.mult)
                q2n = wk.tile([P, 16], f32, tag="q2n")
                nc.vector.tensor_reduce(q2n[:], byq(sc3[:]), AX, OP.add)

                # direct |tf - tgt|^2 -> dall cols 64+
                df = wk.tile([P, 48], f32, tag="df")
                nc.vector.tensor_tensor(df[:], tfn[:], tgN, OP.subtract)
                dfs = wk.tile([P, 48], f32, tag="dfs")
                nc.vector.tensor_tensor(dfs[:], df[:], df[:], OP.mult)
                dd2 = wk.tile([P, 16], f32, tag="dd2")
                nc.vector.tensor_reduce(dd2[:], byq(dfs[:]), AX, OP.add)
                nc.vector.tensor_scalar_max(
                    dall[:, 64 + b * 16 : 64 + (b + 1) * 16], dd2[:], 0.0
                )

                # hi/lo splits, packed for the DMA xbar transpose
                x1 = wk.tile([P, 128], bf16, tag="x1")
                nc.vector.tensor_copy(x1[:, 0:48], tfn[:])
                nc.vector.tensor_tensor(x1[:, 48:96], tfn[:], x1[:, 0:48],
                                        OP.subtract)
                nc.vector.tensor_copy(x1[:, 96:112], q2n[:])
                nc.vector.tensor_tensor(x1[:, 112:128], q2n[:], x1[:, 96:112],
                                        OP.subtract)
                x2 = wk.tile([P, 128], bf16, tag="x2")
                nc.vector.tensor_copy(x2[:, 0:16], r2n[:])
                nc.vector.tensor_tensor(x2[:, 16:32], r2n[:], x2[:, 0:16],
                                        OP.subtract)

                x1t = wk.tile([P, 128], bf16, tag="x1t")
                nc.sync.dma_start(x1t[:], x1[:], transpose=True)
                x2t = wk.tile([P, 128], bf16, tag="x2t")
                nc.sync.dma_start(x2t[:], x2[:], transpose=True)

                lhsT = wk.tile([KR, NP], bf16, tag="lhsT")
                nc.sync.dma_start(rowify(lhsT[0:3, :]), x1t[0:48, :])
                nc.sync.dma_start(rowify(lhsT[3:6, :]), x1t[48:96, :])
                nc.sync.dma_start(rowify(lhsT[6:9, :]), x1t[0:48, :])
                nc.sync.dma_start(rowify(lhsT[9:11, :]), x1t[96:128, :])
                nc.sync.dma_start(lhsT[11:13, :], ones2[:])
                nc.sync.dma_start(rowify(rhs[11:13, :]), x2t[0:32, :])
                return lhsT, rhs

            def scores(b, lhsT, rhs):
                minh = wk.tile([P, 16], f32, tag="minh")
                l1p = None
                l2q = None
                ti = 0  # index among tree-path qts (pair/quad bookkeeping)
                pend = []  # first qt of the pending pair/quad
                for qt in range(16):
                    lhs = lhsT[:, qt * P : (qt + 1) * P]
                    direct = qt in DVE_QTS
                    sb = None
                    dtmp = None
                    if direct:
                        dtmp = wk.tile([P, 2], f32, tag="dtmp")
                    else:
                        sb = wk.tile([P, 2048], bf16, tag="sbq")
                    for h in range(2):
                        s_ps = ps.tile([P, 1024], f32, tag="ps")
                        for j in range(2):
                            c0 = h * 1024 + j * 512
                            nc.tensor.matmul(
                                s_ps[:, j * 512 : (j + 1) * 512],
                                lhs,
                                rhs[:, c0 : c0 + 512],
                                start=True,
                                stop=True,
                            )
                        if direct:
                            nc.vector.tensor_reduce(
                                dtmp[:, h : h + 1], s_ps[:], AX, OP.min
                            )
                        else:
                            nc.scalar.copy(
                                sb[:, h * 1024 : (h + 1) * 1024], s_ps[:]
                            )
                    if direct:
                        nc.vector.tensor_reduce(
                            minh[:, qt : qt + 1], dtmp[:], AX, OP.min
                        )
                        continue
                    # lvl1 per qt: 2048 -> 1024, pair-packed
                    slot = ti % 2
                    if slot == 0:
                        l1p = wk.tile([P, 2048], bf16, tag="l1p")
                        pend.append(qt)
                    nc.vector.tensor_tensor(
                        l1p[:, slot * 1024 : (slot + 1) * 1024],
                        sb[:, 0:1024], sb[:, 1024:2048], OP.min,
                    )
                    if slot == 1:
                        # lvl2 batched over the qt pair -> two 512 blocks
                        blk = (ti // 2) % 2
                        if blk == 0:
                            l2q = wk.tile([P, 2048], bf16, tag="l2q")
                        v1 = l1p[:].rearrange("p (a c) -> p a c", a=2)
                        nc.vector.tensor_tensor(
                            l2q[:, blk * 1024 : (blk + 1) * 1024].rearrange(
                                "p (a c) -> p a c", a=2
                            ),
                            v1[:, :, 0:512], v1[:, :, 512:1024], OP.min,
                        )
                        if blk == 1:
                            # lvl3/lvl4/final reduce batched over the quad;
                            # tree qts of this quad are contiguous (q0..q0+3)
                            q0 = pend[0]
                            v2 = l2q[:].rearrange("p (a c) -> p a c", a=4)
                            l3 = wk.tile([P, 1024], bf16, tag="l3")
                            nc.vector.tensor_tensor(
                                l3[:].rearrange("p (a c) -> p a c", a=4),
                                v2[:, :, 0:256], v2[:, :, 256:512], OP.min,
                            )
                            v3 = l3[:].rearrange("p (a c) -> p a c", a=4)
                            l4 = wk.tile([P, 512], bf16, tag="l4")
                            nc.vector.tensor_tensor(
                                l4[:].rearrange("p (a c) -> p a c", a=4),
                                v3[:, :, 0:128], v3[:, :, 128:256], OP.min,
                            )
                            nc.vector.tensor_reduce(
                                minh[:, q0 : q0 + 4],
                                l4[:].rearrange("p (a c) -> p a c", a=4),
                                AX, OP.min,
                            )
                            pend = []
                    ti += 1
                if pend:
                    # leftover pair: lvl3/lvl4/reduce at pair granularity
                    q0 = pend[0]
                    v2 = l2q[:, 0:1024].rearrange("p (a c) -> p a c", a=2)
                    l3 = wk.tile([P, 1024], bf16, tag="l3")
                    nc.vector.tensor_tensor(
                        l3[:, 0:512].rearrange("p (a c) -> p a c", a=2),
                        v2[:, :, 0:256], v2[:, :, 256:512], OP.min,
                    )
                    v3 = l3[:, 0:512].rearrange("p (a c) -> p a c", a=2)
                    l4 = wk.tile([P, 512], bf16, tag="l4")
                    nc.vector.tensor_tensor(
                        l4[:, 0:256].rearrange("p (a c) -> p a c", a=2),
                        v3[:, :, 0:128], v3[:, :, 128:256], OP.min,
                    )
                    nc.vector.tensor_reduce(
                        minh[:, q0 : q0 + 2],
                        l4[:, 0:256].rearrange("p (a c) -> p a c", a=2),
                        AX, OP.min,
                    )
                nc.vector.tensor_scalar_max(
                    dall[:, b * 16 : (b + 1) * 16], minh[:], 0.0
                )

            handles = prep(0)
            for b in range(BPC):
                nxt = prep(b + 1) if b + 1 < BPC else None
                scores(b, *handles)
                handles = nxt

            # ---- sqrt then mean over the 2048 points of each batch
            sq = acc.tile([P, P], f32, tag="sq")
            nc.scalar.activation(sq[:], dall[:], AF.Sqrt)
            o_ps = ps.tile([P, 128], f32, tag="ps")
            nc.tensor.matmul(o_ps[0:1, 0:P], wvec[:], sq[:], start=True, stop=True)
            osb = acc.tile([1, 8], f32, tag="osb")
            nc.vector.tensor_reduce(
                osb[:], o_ps[0:1, 0:P].rearrange("p (a c) -> p a c", c=16), AX, OP.add
            )
            nc.sync.dma_start(out.ap(), osb[:])

    nc.compile()
    return nc


def _host_prep(target, model_points, idx, H):
    """Per-core input maps. Layout/dtype transforms + O(B) H/idx math only."""
    import ml_dtypes

    bf = ml_dtypes.bfloat16
    tgt = np.ascontiguousarray(target, dtype=np.float32)
    mp = np.ascontiguousarray(model_points, dtype=np.float32)
    Hf = np.ascontiguousarray(H, dtype=np.float32)

    # rhs host rows: hi/lo split of -2*target coords (planar rows)
    t2 = (-2.0 * tgt).transpose(0, 2, 1)  # [BS, 3, NP]
    t2hi = t2.astype(bf)
    t2lo = (t2 - t2hi.astype(np.float32)).astype(bf)
    rhsA = np.zeros((BS, 11, NP), dtype=bf)
    rhsA[:, 0:3] = t2hi
    rhsA[:, 3:6] = t2hi
    rhsA[:, 6:9] = t2lo
    rhsA[:, 9:11] = np.ones((1, 2, NP), dtype=bf)

    # planar natural q-major layout: col 16c+q holds point 128q+p, channel c
    def nat(x):  # [BS, NP, 3] -> [BS, P, 48] planar
        return (
            x.reshape(BS, 16, P, 3).transpose(0, 2, 3, 1).reshape(BS, P, 48)
        )

    pk2 = np.zeros((BS, P, 160), dtype=np.float32)
    pk2[:, :, 0:48] = nat(mp)
    pk2[:, :, 48:96] = nat(tgt)
    t_vec = Hf[:, :3, 3]
    pk2[:, :, 96:144] = np.repeat(t_vec, 16, axis=1)[:, None, :]
    # bbc col 3d+e = base[d,e] = H[e,d]
    bb = Hf[:, :3, :3].transpose(0, 2, 1).reshape(BS, 9)
    pk2[:, :, 144:153] = bb[:, None, :]

    onesr = np.ones((2, NP), dtype=bf)
    wvv = np.full((P, 1), 1.0 / NP, dtype=np.float32)

    in_maps = []
    for c in range(NCORES):
        sl = slice(c * BPC, (c + 1) * BPC)
        in_maps.append(
            {
                "rhsA": np.ascontiguousarray(rhsA[sl]),
                "pk2": np.ascontiguousarray(pk2[sl]),
                "onesr": onesr,
                "wv": wvv,
            }
        )
    is_sym = (np.asarray(idx)[:, 0] % 2 == 0) & (np.asarray(idx)[:, 0] < 16)
    return in_maps, is_sym


def run_on_device(target, model_points, idx, H, trace=False, **kw):
    from concourse import bass_utils

    nc = _build()
    in_maps, is_sym = _host_prep(target, model_points, idx, H)
    res = bass_utils.run_bass_kernel_spmd(
        nc, in_maps, core_ids=list(range(NCORES)), trace=trace, **kw
    )
    dis = np.zeros((BS,), dtype=np.float32)
    for c in range(NCORES):
        o = res.results[c]["out"].reshape(8)
        for bb_ in range(BPC):
            g = c * BPC + bb_
            dis[g] = o[bb_] if is_sym[g] else o[4 + bb_]
    return dis, res


def kernel(target, model_points, idx, H):
    dis, _ = run_on_device(target, model_points, idx, H)
    return dis
